# revision 25
# baseline (speedup 1.0000x reference)
"""GCN (message-passing) Trainium2 Bass kernel, 8-core SPMD.

out = relu(scatter_add(norm * (x @ W_lin.T + b_lin)[src], dst) + x @ W_root.T + b_root)
with norm = dinv[src]*dinv[dst], dinv = rsqrt(max(in_degree, 1)).

Strategy (dst-sharding, full input replication):
  - Host: partition edges by dst owner core (6250 nodes/core), sort by
    (src-bucket, dst-block, src), pad each (bucket, dst-block) segment to a
    multiple of 128 edges using a schedule shared across cores (max over
    cores), precompute index-derived scalars (dinv, c = dinv*sum(dinv[src])).
  - Device, per core: dma_gather rows of the dinv-prescaled x table (padded
    to 512B rows; int16 indices force a 2-bucket split of the node table at
    32768); per 128-edge tile build a one-hot S matrix (iota == dst_local) on
    DVE and accumulate A^T[96, 128] = sum_tiles Xg^T S in PSUM on the PE; per
    dst block scale by dinv[dst], then two small matmuls fold
    W_lin/W_root/b_lin/b_root/relu into the final [128, 96] output block.
"""

import sys

import numpy as np

# concourse (Bass/Tile) lives in the container's trn_rl_repo checkout; make
# kernel.py importable from any working directory.
for _p in ("/opt/trn_rl_repo", "/root/.axon_site/_ro/trn_rl_repo"):
    if _p not in sys.path:
        sys.path.insert(0, _p)

N_CORES = 8
D = 96
ELEM = 128           # padded gather row: 128 f32 = 512 B
BLK = 128            # dst nodes per block
BUCKET = 32768       # int16 index limit -> split node table
CT = 8               # gather chunk size in 128-edge tiles (<=1024 idx/call)
DMA_SCRATCH = 16384  # SWDGE descriptor carveout (bytes/partition); 16 B/desc


def _cdiv(a, b):
    return (a + b - 1) // b


def _prep(x, edge_index):
    """Host-side sharding/layout. Returns per-core input arrays + schedule."""
    N = x.shape[0]
    NPC = N // N_CORES
    NBLK = _cdiv(NPC, BLK)
    src = edge_index[0].astype(np.int64)
    dst = edge_index[1].astype(np.int64)

    deg = np.bincount(dst, minlength=N).astype(np.float32)
    dinv = (1.0 / np.sqrt(np.maximum(deg, 1.0))).astype(np.float32)
    w = np.zeros(N, np.float32)
    np.add.at(w, dst, dinv[src])
    c = (dinv * w).astype(np.float32)

    xs = np.zeros((N, ELEM), np.float32)
    xs[:, :D] = x * dinv[:, None]

    # Degree-balanced dst relabeling: deal nodes (sorted by in-degree) cyclically
    # across the (core, block) bins so every block has ~equal edge count. This
    # equalizes the shared max-over-cores tile schedule, cutting pad descriptors
    # on the Pool engine (the kernel's bottleneck). perm[newpos] = orig node.
    nbins = N_CORES * NBLK
    cap = np.full(nbins, BLK, np.int64)
    cap[NBLK - 1::NBLK] = NPC - (NBLK - 1) * BLK
    order_nodes = np.argsort(-deg, kind="stable")
    perm = np.empty(N, np.int64)
    fill = np.zeros(nbins, np.int64)
    base = np.arange(N_CORES)[:, None] * NPC + np.arange(NBLK)[None, :] * BLK
    base = base.reshape(-1)
    bi = 0
    for nd in order_nodes:
        while fill[bi] >= cap[bi]:
            bi = (bi + 1) % nbins
        perm[base[bi] + fill[bi]] = nd
        fill[bi] += 1
        bi = (bi + 1) % nbins
    invp = np.empty(N, np.int64)
    invp[perm] = np.arange(N)
    dstn = invp[dst]

    cores = []
    counts = np.zeros((N_CORES, 2, NBLK), np.int64)
    for cc in range(N_CORES):
        m = (dstn >= cc * NPC) & (dstn < (cc + 1) * NPC)
        s = src[m]
        dl = dstn[m] - cc * NPC
        bk = (s >= BUCKET).astype(np.int64)
        blk = dl // BLK
        order = np.lexsort((s, blk, bk))
        s, dl, bk, blk = s[order], dl[order], bk[order], blk[order]
        cores.append((s, dl, bk, blk))
        for k in range(2):
            counts[cc, k] = np.bincount(blk[bk == k], minlength=NBLK)

    # shared tile schedule: tiles per (bucket, block) = max over cores
    T = _cdiv(counts, BLK).max(axis=0)          # [2, NBLK]
    seg_off = np.zeros((2, NBLK), np.int64)     # tile offset of each segment
    flat = T.reshape(-1)
    seg_off.reshape(-1)[1:] = np.cumsum(flat)[:-1]
    t_total = int(flat.sum())
    L = t_total * BLK

    per_core = []
    for cc in range(N_CORES):
        s, dl, bk, blk = cores[cc]
        gidx_flat = np.zeros(L, np.int16)       # pad slots gather row 0 (valid)
        dloc_flat = np.full(L, -1, np.float32)  # pad slots produce zero S rows
        pos = 0
        for k in range(2):
            for b in range(NBLK):
                n = counts[cc, k, b]
                o = seg_off[k, b] * BLK
                gidx_flat[o:o + n] = (s[pos:pos + n] - BUCKET * k).astype(np.int16)
                dloc_flat[o:o + n] = (dl[pos:pos + n] - b * BLK).astype(np.float32)
                pos += n
        gidx16 = gidx_flat.reshape(L // 16, 16).T       # slot i -> [i%16, i//16]
        gidx = np.tile(gidx16, (8, 1)).copy()           # replicate for 8 gpsimd cores
        dloc = dloc_flat.reshape(t_total, BLK).T.copy() # slot i -> [i%128, i//128]

        own = perm[cc * NPC:(cc + 1) * NPC]
        xroot = np.empty((D + 2, NPC), np.float32)
        xroot[:D] = x[own].T
        xroot[D] = 1.0
        xroot[D + 1] = c[own]
        dinvb = np.broadcast_to(dinv[own], (D, NPC)).copy()
        per_core.append({"gidx": gidx, "dloc": dloc, "xroot": xroot, "dinvb": dinvb})

    sched = {"N": N, "NPC": NPC, "NBLK": NBLK, "T": T, "seg_off": seg_off,
             "t_total": t_total, "L": L, "perm": perm,
             "ta_total": int(T[0].sum()), "tb_total": int(T[1].sum())}
    return xs, per_core, sched


def _build(sched):
    import concourse.bacc as bacc
    import concourse.tile as tile
    from concourse import mybir, library_config

    N, NPC, NBLK = sched["N"], sched["NPC"], sched["NBLK"]
    T, seg_off, t_total, L = sched["T"], sched["seg_off"], sched["t_total"], sched["L"]
    bucket_tiles = [sched["ta_total"], sched["tb_total"]]
    bucket_tile0 = [0, sched["ta_total"]]

    f32, i32, i16 = mybir.dt.float32, mybir.dt.int32, mybir.dt.int16
    eq, mx, mult = (mybir.AluOpType.is_equal, mybir.AluOpType.max,
                    mybir.AluOpType.mult)

    nc = bacc.Bacc("TRN2", target_bir_lowering=False, debug=False,
                   num_devices=N_CORES, num_swdge_queues=4,
                   dynamic_dma_scratch_size=DMA_SCRATCH)
    xs = nc.dram_tensor("xs", [N, ELEM], f32, kind="ExternalInput").ap()
    gidx = nc.dram_tensor("gidx", [128, L // 16], i16, kind="ExternalInput").ap()
    dloc = nc.dram_tensor("dloc", [128, t_total], f32, kind="ExternalInput").ap()
    xroot = nc.dram_tensor("xroot", [D + 2, NPC], f32, kind="ExternalInput").ap()
    dinvb = nc.dram_tensor("dinvb", [D, NPC], f32, kind="ExternalInput").ap()
    wlin = nc.dram_tensor("wlin", [D, D], f32, kind="ExternalInput").ap()
    wroot = nc.dram_tensor("wroot", [D + 2, D], f32, kind="ExternalInput").ap()
    iota = nc.dram_tensor("iota", [128, BLK], f32, kind="ExternalInput").ap()
    outp = nc.dram_tensor("out", [NPC, D], f32, kind="ExternalOutput").ap()

    xs_view = [xs[0:BUCKET, :], xs[BUCKET:N, :]]

    with tile.TileContext(nc) as tc:
        nc.gpsimd.load_library(library_config.mlp)
        with (
            tc.tile_pool(name="const", bufs=1) as cpool,
            tc.tile_pool(name="xga", bufs=10) as xga_pool,
            tc.tile_pool(name="xgb", bufs=10) as xgb_pool,
            tc.tile_pool(name="s", bufs=24) as s_pool,
            tc.tile_pool(name="asb", bufs=4) as asb_pool,
            tc.tile_pool(name="outt", bufs=4) as out_pool,
            tc.tile_pool(name="psA", bufs=5, space="PSUM") as psA_pool,
            tc.tile_pool(name="psB", bufs=3, space="PSUM") as psB_pool,
        ):
            iota_t = cpool.tile([128, BLK], f32)
            gidx_t = cpool.tile([128, L // 16], i16)
            dloc_t = cpool.tile([128, t_total], f32)
            xroot_t = cpool.tile([D + 2, NPC], f32)
            dinvb_t = cpool.tile([D, NPC], f32)
            wlin_t = cpool.tile([D, D], f32)
            wroot_t = cpool.tile([D + 2, D], f32)
            # split the gidx load so the first gathers only wait on slice 0;
            # load gather-critical data first on the sync queue, big epilogue
            # consts on the scalar engine's HWDGE queue in parallel
            GW = L // 16
            gsplit = [0, GW // 32, GW // 8, GW // 4, GW // 2, GW]
            for a0, a1 in zip(gsplit[:-1], gsplit[1:]):
                nc.sync.dma_start(out=gidx_t[:, a0:a1], in_=gidx[:, a0:a1])
            nc.sync.dma_start(out=dloc_t[:], in_=dloc)
            nc.sync.dma_start(out=iota_t[:], in_=iota)
            for t, a in ((xroot_t, xroot), (dinvb_t, dinvb), (wlin_t, wlin),
                         (wroot_t, wroot)):
                nc.sync.dma_start(out=t[:], in_=a)

            # gather chunk tiles per bucket (created lazily in stream order)
            chunks = [[], []]

            def ensure_chunk(k, ci):
                while len(chunks[k]) <= ci:
                    j = len(chunks[k])
                    t0 = j * CT
                    ct = min(CT, bucket_tiles[k] - t0)
                    pool = xga_pool if k == 0 else xgb_pool
                    xt = pool.tile([128, CT, ELEM], f32, tag=f"xg{k}")
                    slot0 = (bucket_tile0[k] + t0) * BLK
                    n = ct * BLK
                    nc.gpsimd.dma_gather(
                        xt[:, 0:ct, :], xs_view[k],
                        gidx_t[:, slot0 // 16:(slot0 + n) // 16],
                        n, n, ELEM, queue_num=(2 * k + j) % 4)
                    chunks[k].append(xt)
                return chunks[k][ci]

            for b in range(NBLK):
                bs = b * BLK
                rows = min(BLK, NPC - bs)
                n_tiles = int(T[0][b] + T[1][b])
                psA = (psA_pool.tile([D, BLK], f32, name="psA", tag="psA")
                       if n_tiles else None)
                ki = 0
                for k in range(2):
                    for t in range(int(T[k][b])):
                        gg = int(seg_off[k][b]) + t         # global stream idx
                        g = gg - bucket_tile0[k]            # bucket-rel tile idx
                        xt = ensure_chunk(k, g // CT)
                        S = s_pool.tile([128, BLK], f32)
                        nc.vector.tensor_tensor(
                            out=S[:], in0=iota_t[:],
                            in1=dloc_t[:, gg:gg + 1].to_broadcast([128, BLK]),
                            op=eq)
                        nc.tensor.matmul(
                            out=psA[:, :], lhsT=xt[:, g % CT, 0:D], rhs=S[:],
                            start=(ki == 0), stop=(ki == n_tiles - 1))
                        ki += 1

                psB = psB_pool.tile([BLK, D], f32)
                if n_tiles:
                    asb = asb_pool.tile([D, BLK], f32)
                    nc.vector.tensor_tensor(
                        out=asb[:, 0:rows], in0=psA[:, 0:rows],
                        in1=dinvb_t[:, bs:bs + rows], op=mult)
                    nc.tensor.matmul(out=psB[0:rows, :], lhsT=asb[:, 0:rows],
                                     rhs=wlin_t[:], start=True, stop=False)
                    nc.tensor.matmul(out=psB[0:rows, :],
                                     lhsT=xroot_t[:, bs:bs + rows],
                                     rhs=wroot_t[:], start=False, stop=True)
                else:
                    nc.tensor.matmul(out=psB[0:rows, :],
                                     lhsT=xroot_t[:, bs:bs + rows],
                                     rhs=wroot_t[:], start=True, stop=True)
                ot = out_pool.tile([BLK, D], f32)
                nc.vector.tensor_scalar(ot[0:rows, :], psB[0:rows, :], 0.0,
                                        None, mx)
                nc.sync.dma_start(out=outp[bs:bs + rows, :], in_=ot[0:rows, :])

    nc.compile()
    return nc


def kernel(x, edge_index, W_lin, b_lin, W_root, b_root):
    from concourse.bass_utils import run_bass_kernel_spmd

    x = np.asarray(x, dtype=np.float32)
    edge_index = np.asarray(edge_index)
    W_lin = np.asarray(W_lin, np.float32)
    b_lin = np.asarray(b_lin, np.float32)
    W_root = np.asarray(W_root, np.float32)
    b_root = np.asarray(b_root, np.float32)

    xs, per_core, sched = _prep(x, edge_index)
    nc = _build(sched)

    wlin_in = W_lin.T.copy()
    wroot_in = np.empty((D + 2, D), np.float32)
    wroot_in[:D] = W_root.T
    wroot_in[D] = b_root
    wroot_in[D + 1] = b_lin
    iota_in = np.broadcast_to(np.arange(BLK, dtype=np.float32), (128, BLK)).copy()

    in_maps = []
    for cc in range(N_CORES):
        pc = per_core[cc]
        in_maps.append({
            "xs": xs, "gidx": pc["gidx"], "dloc": pc["dloc"],
            "xroot": pc["xroot"], "dinvb": pc["dinvb"],
            "wlin": wlin_in, "wroot": wroot_in, "iota": iota_in,
        })
    res = run_bass_kernel_spmd(nc, in_maps, core_ids=list(range(N_CORES)))
    shards = np.concatenate([res.results[cc]["out"] for cc in range(N_CORES)],
                            axis=0)
    out = np.empty_like(shards)
    out[sched["perm"]] = shards          # undo the dst relabeling
    return out
